# revision 48
# baseline (speedup 1.0000x reference)
"""DeformableAttention1D on 8 TRN2 NeuronCores via Bass/Tile.

Sharding: core c handles offset-group g=c//2 (64 of 256 channels, 2 of 8 heads)
and query-half qh=c%2 (512 of 1024 positions). Each core computes its group's
offsets/gather/CPB/attention independently; the final output projection is
computed as a partial (wo sliced by group) and summed on the host.

Key idea vs the one-hot/MLP baseline: both the grid_sample gather AND the CPB
relative-position-bias MLP are evaluated via SWDGE dma_gather from
host-precomputed DRAM tables.

  * kv gather: rows of x^T (zero-padded, pairs [x_i | x_{i+1}]) indexed by
    floor(pixel coord); bilinear lerp is 2 DVE ops with per-partition weights.
  * CPB bias: bias(q,j,o) = G_o(pos) with pos = grid_q[q] - vgs[j] and G_o a
    fixed scalar function of the CPB weights only. grid_q is a uniform grid
    with spacing delta = 2/1023, so for fixed j the 512 query positions read a
    CONTIGUOUS window of a delta-spaced table of G_o. One dma_gather of 256
    windowed rows (fp16) + a per-partition lerp replaces the whole MLP.
    (b3 is dropped: constant per (o,q) shift cancels in softmax.)

The ACT engine is restricted to ONE table set (exp_and_others: Exp, Tanh,
Square, Copy, Relu, ...); gelu uses the tanh approximation natively.
"""
import os
import sys

sys.path.insert(0, "/opt/trn_rl_repo")

DEBUG = bool(os.environ.get("DEFORM_DEBUG"))

import numpy as np

import concourse.bacc as bacc
import concourse.bass as bass
import concourse.mybir as mybir
import concourse.tile as tile
import concourse.bass_utils as bass_utils

F32 = mybir.dt.float32
F32R = mybir.dt.float32r
F16 = mybir.dt.float16
BF16 = mybir.dt.bfloat16
I32 = mybir.dt.int32
I16 = mybir.dt.int16
U32 = mybir.dt.uint32
AF = mybir.ActivationFunctionType
ALU = mybir.AluOpType

# model dims (hardcoded per problem spec)
DIM = 256
N = 1024
G = 4
HEADS = 8
DH = 32
NDS = 256          # downsampled kv positions
QS = 512           # queries per core
DPG = 64           # channels per group
OFF_K = 6
DS = 4             # downsample stride
OFF_SCALE = 4.0
NCORES = 8

DELTA = 2.0 / 1023.0
POS0 = -2.05
K2 = 1023.0 / 255.0
WIN = 512          # CPB table row length (stride-2 slice of delta/2 grid)
PMAX = 3200        # CPB windowed-table rows
TLEN = PMAX + 2 * WIN  # underlying table length (delta/2 spacing)
XROWS = 1059       # kv table rows (pairs), indexed by floor(ppix)+17

# A&S 7.1.26 erf coefficients (|err| <= 1.5e-7)
ERF_P = 0.3275911
ERF_A = [0.254829592, -0.284496736, 1.421413741, -1.453152027, 1.061405429]

# packed_a (f32r, [64, 516], conv-critical): wtaps 0:384, wqT 384:448,
#   wproj 448:449, bodw row (p0) 449:513.
# packed_b (f32r, [64, 384]): wkTs 0:64, wvT 64:128, woT 128:384.
# rowA2c/rowB2n (index affine rows) ship via the separate [1,512] "rows2"
# input (DVE TSP requires equal base partitions for its SB tensor inputs).
PKA_C = 516
PKB_C = 384

_CACHED = {}


def _patch_act_tables():
    """Restrict activation-table selection to the single set that covers all
    ACT functions used by this kernel, so exactly one table load is emitted."""
    import concourse.hw_specs as hw_specs

    if getattr(bacc, "_deform_act_patch", False):
        return
    orig = hw_specs.get_activation_tables

    keep = "exp_and_others"

    def patched(module_arch):
        tabs = orig(module_arch)
        keep_funcs = tabs[keep]
        out = {}
        for name, funcs in tabs.items():
            if name == keep:
                out[name] = funcs
            else:
                out[name] = funcs - keep_funcs
        return out

    bacc.get_activation_tables = patched
    bacc._deform_act_patch = True


def build_nc():
    _patch_act_tables()
    nc = bacc.Bacc("TRN2", target_bir_lowering=False, debug=False, num_devices=NCORES)

    din = {}

    def dt_in(name, shape, dtype=F32):
        din[name] = nc.dram_tensor(name, shape, dtype, kind="ExternalInput")
        return din[name]

    dt_in("xg", [DPG, N], F32R)
    dt_in("xq", [DPG, QS], F32R)
    dt_in("packed_a", [DPG, PKA_C], F32R)
    dt_in("packed_b", [DPG, PKB_C], F32R)
    dt_in("rows2", [16, 288], F32)
    dt_in("cpb_tab", [PMAX, 2 * WIN], F16)
    dt_in("xt2", [XROWS, 2 * DPG], F32)
    idx_scr = nc.dram_tensor("idx_scr", [1, NDS], F32, kind="Internal")
    y_out = nc.dram_tensor("y", [128, 2 * QS], F16, kind="ExternalOutput")
    dbg = {}
    if DEBUG:
        for nm, shp in [("dbg_conv", [DPG, NDS]), ("dbg_gl", [DPG, NDS]),
                        ("dbg_r", [1, NDS]), ("dbg_T2", [1, 2 * NDS]),
                        ("dbg_P2", [1, 2 * NDS]), ("dbg_idx", [16, 32]),
                        ("dbg_kv", [DPG, NDS]), ("dbg_k", [DPG, NDS]),
                        ("dbg_bias00", [128, QS]), ("dbg_logit00", [128, QS]),
                        ("dbg_avn", [DPG, QS])]:
            dbg[nm] = nc.dram_tensor(nm, shp, F32, kind="ExternalOutput")

    qh_off = 1  # xgp column offset of x (left zero pad)

    with tile.TileContext(nc) as tc:
        with (
            tc.tile_pool(name="const", bufs=1) as cst,
            tc.tile_pool(name="work", bufs=2) as wk,
            tc.tile_pool(name="rows", bufs=1) as rw,
            tc.tile_pool(name="pers", bufs=1) as pe_pool,
        ):
            # ---------- t=0: idle-engine prep ----------
            xgp = cst.tile([DPG, N + 4], F32R, name="xgp", tag="xgp")
            nc.gpsimd.memset(xgp[:, 0:1].bitcast(F32), 0.0)
            nc.gpsimd.memset(xgp[:, 1 + N:N + 4].bitcast(F32), 0.0)
            idx16 = cst.tile([128, 32], I16, name="idx16", tag="idx16")
            # tiled identity [16, 128]: eye16[c, j] = (j % 16 == c), for
            # replicating the idx block to all 8 Q7 16-partition groups
            eyeio16 = cst.tile([16, 128], I32, name="eyeio16", tag="eyeio16")
            nc.gpsimd.iota(eyeio16[:], pattern=[[0, 8], [1, 16]], base=0,
                           channel_multiplier=-1)
            eye16 = cst.tile([16, 128], F32, name="eye16", tag="eye16")
            nc.vector.tensor_scalar(eye16[:], eyeio16[:], 0, None, ALU.is_equal)
            ones_row = cst.tile([1, NDS], F32R, name="ones_row", tag="ones_row")
            nc.gpsimd.memset(ones_row[:].bitcast(F32), 1.0)
            ones_colf = cst.tile([128, 1], F32, name="ones_colf", tag="ones_colf")
            nc.gpsimd.memset(ones_colf[:], 1.0)
            ones_col = cst.tile([128, 1], BF16, name="ones_col", tag="ones_col")
            nc.vector.tensor_copy(ones_col[:], ones_colf[:])
            # identity for PE transposes (f32)
            eyeio = cst.tile([128, 128], I32, name="eyeio", tag="eyeio")
            nc.gpsimd.iota(eyeio[:], pattern=[[1, 128]], base=0, channel_multiplier=-1)
            eyef = cst.tile([128, 128], F32, name="eyef", tag="eyef")
            nc.vector.tensor_scalar(eyef[:], eyeio[:], 0, None, ALU.is_equal)
            # warm the single ACT table at t=0 (overlaps input DMAs)
            wsrc = cst.tile([128, 1], F32, name="wsrc", tag="wsrc")
            nc.gpsimd.memset(wsrc[:], 0.0)
            warm = cst.tile([128, 1], F32, name="warm", tag="warm")
            nc.scalar.activation(warm[:], wsrc[:], AF.Relu)
            # PE p-state warmup fodder
            wmm = cst.tile([128, 128], F32R, name="wmm", tag="wmm")
            nc.gpsimd.memset(wmm[:].bitcast(F32), 0.0)

            # ---------- input DMAs (packed on ACT queue; rest on SP) ----
            packed_a = cst.tile([DPG, PKA_C], F32R, name="packed_a", tag="packed_a")
            packed_b = cst.tile([DPG, PKB_C], F32R, name="packed_b", tag="packed_b")
            with tc.high_priority():
                nc.scalar.dma_start(packed_a[:], din["packed_a"].ap())
                nc.sync.dma_start(xgp[:, qh_off:qh_off + N], din["xg"].ap())
            nc.scalar.dma_start(packed_b[:], din["packed_b"].ap())
            xqt = cst.tile([DPG, QS], F32R, name="xqt", tag="xqt")
            nc.sync.dma_start(xqt[:], din["xq"].ap())
            rows2 = cst.tile([16, 288], F32, name="rows2", tag="rows2")
            nc.sync.dma_start(rows2[:], din["rows2"].ap())
            wtaps = packed_a[0:DPG, 0:384]
            wqT = packed_a[0:DPG, 384:448]
            wproj = packed_a[0:DPG, 448:449]
            bodw_row = packed_a[0:1, 449:513]
            wkTs = packed_b[0:DPG, 0:64]
            wvT = packed_b[0:DPG, 64:128]
            woT = packed_b[0:DPG, 128:384]
            rowA2w = rows2[0:16, 0:16]
            rowB2w = rows2[0:16, 16:32]
            rowB2r = rows2[0:1, 32:288]

            # persistent tiles crossing phases
            qs_sb = pe_pool.tile([DPG, QS], F32R, name="qs_sb", tag="qs_sb")
            k_sb = pe_pool.tile([DPG, NDS], F32R, name="k_sb", tag="k_sb")
            kv_sb = pe_pool.tile([DPG, NDS], F32R, name="kv_sb", tag="kv_sb")
            vT = [pe_pool.tile([128, DPG], BF16, name=f"vT{H}", tag=f"vT{H}")
                  for H in range(2)]
            fw = pe_pool.tile([128, 2], F32, name="fw", tag="fw")
            cpbg = pe_pool.tile([128, 2 * 2 * WIN], F16, name="cpbg", tag="cpbg")
            kvg = pe_pool.tile([128, 2 * 2 * DPG], F32, name="kvg", tag="kvg")
            avn = pe_pool.tile([DPG, QS], F32R, name="avn", tag="avn")

            with tc.tile_pool(name="psA", bufs=1, space="PSUM") as psA:
                # ---------- conv (strided depthwise fused with wq) ----------
                pconv = psA.tile([DPG, NDS], F32, name="pconv", tag="pconv")
                # PE clock warmup: dependency-free matmuls keep the ramp model
                # at full speed by the time real matmuls arrive
                for w in range(16):
                    nc.tensor.matmul(pconv[0:DPG, 0:64], wmm[:, 0:DPG],
                                     wmm[:, 0:64], skip_group_check=True)
                for k in range(OFF_K):
                    nc.tensor.matmul(
                        pconv[:], wtaps[:, 64 * k:64 * k + 64],
                        xgp[:, k:k + DS * (NDS - 1) + 1:DS],
                        start=(k == 0), stop=False)
                nc.tensor.matmul(pconv[:], bodw_row, ones_row[:],
                                 start=False, stop=True)
                if DEBUG:
                    dcv = wk.tile([DPG, NDS], F32, name="dcv", tag="dcv")
                    nc.vector.tensor_copy(dcv[:], pconv[:])
                    nc.sync.dma_start(dbg["dbg_conv"].ap(), dcv[:])

                # ---------- gelu (tanh approx, native ACT tanh) ----------
                # 2*gelu(x) = x * (1 + tanh(c1*(x + c2*x^3)))
                sq = wk.tile([DPG, NDS], F32, name="g_sq", tag="g_sq")
                nc.scalar.activation(sq[:], pconv[:], AF.Square)
                x3 = wk.tile([DPG, NDS], F32, name="g_x3", tag="g_x3")
                nc.vector.tensor_tensor(x3[:], sq[:], pconv[:], ALU.mult)
                arg = wk.tile([DPG, NDS], F32, name="g_arg", tag="g_arg")
                nc.vector.scalar_tensor_tensor(arg[:], x3[:], 0.044715, pconv[:],
                                               ALU.mult, ALU.add)
                tg = wk.tile([DPG, NDS], F32, name="g_tg", tag="g_tg")
                nc.scalar.activation(tg[:], arg[:], AF.Tanh,
                                     scale=0.7978845608028654)
                gl = wk.tile([DPG, NDS], F32R, name="g_gl", tag="g_gl")
                nc.vector.scalar_tensor_tensor(gl[:], tg[:], 1.0, pconv[:],
                                               ALU.add, ALU.mult)
                if DEBUG:
                    nc.sync.dma_start(dbg["dbg_gl"].ap(), gl[:].bitcast(F32))

                # ---------- proj + tanh (as r = 1/(e^{2p}+1)) ----------
                pproj = psA.tile([1, NDS], F32, name="pproj", tag="pproj")
                nc.tensor.matmul(pproj[:], wproj, gl[:])
                # qs early on PE (data ready; overlaps the row chain)
                pqs = psA.tile([DPG, QS], F32, name="pqs", tag="pqs")
                nc.tensor.matmul(pqs[:], wqT, xqt[:])
                nc.scalar.copy(qs_sb[:], pqs[:])

                pjr = rw.tile([1, NDS], F32, name="pjr", tag="pjr")
                nc.scalar.copy(pjr[:], pproj[:])
                # fire the proj-row wrap round-trip immediately (SP queue);
                # tanh + index math happen post-wrap on [16, 16] tiles; the kv
                # fraction row overlaps the round-trip.
                nc.sync.dma_start(idx_scr.ap(), pjr[:])
                pjw = rw.tile([16, 16], F32, name="pjw", tag="pjw")
                nc.sync.dma_start(
                    pjw[:], idx_scr.ap().rearrange("a (s p) -> (a p) s", p=16))
                th = rw.tile([1, NDS], F32, name="th", tag="th")
                nc.scalar.activation(th[:], pproj[:], AF.Tanh)
                if DEBUG:
                    nc.sync.dma_start(dbg["dbg_r"].ap(), th[:])

                # row side (overlapped): ppix row -> floor -> fraction -> fw
                PXr = rw.tile([1, NDS], F32, name="PXr", tag="PXr")
                nc.vector.scalar_tensor_tensor(
                    PXr[:], th[:], float(4096.0 / 255.0), rowB2r,
                    ALU.mult, ALU.add)
                XI = rw.tile([1, NDS], I32, name="XI", tag="XI")
                nc.vector.tensor_copy(XI[:], PXr[:])
                XC = rw.tile([1, NDS], F32, name="XC", tag="XC")
                nc.vector.tensor_copy(XC[:], XI[:])
                XG = rw.tile([1, NDS], F32, name="XG", tag="XG")
                nc.vector.tensor_tensor(XG[:], XC[:], PXr[:], ALU.is_gt)
                XP = rw.tile([1, NDS], F32, name="XP", tag="XP")
                nc.vector.tensor_tensor(XP[:], XC[:], XG[:], ALU.subtract)
                F2 = rw.tile([1, NDS], F32, name="F2", tag="F2")
                nc.vector.tensor_tensor(F2[:], PXr[:], XP[:], ALU.subtract)

                # kv lerp weights to per-partition columns: fw = [w1_H0, w1_H1]
                ptf = psA.tile([128, 2], F32, name="ptf", tag="ptf")
                for H in range(2):
                    nc.tensor.transpose(ptf[:, H:H + 1],
                                        F2[0:1, 128 * H:128 * (H + 1)],
                                        eyef[0:1, 0:1])
                nc.scalar.copy(fw[:], ptf[:])

                # wrapped side: u2/ppix affine on [16, 16], floors on [16, 32],
                # then replicate to all 8 Q7 core groups via PE matmul
                thw = rw.tile([16, 16], F32, name="thw", tag="thw")
                nc.scalar.activation(thw[:], pjw[:], AF.Tanh)
                UW = rw.tile([16, 32], F32, name="UW", tag="UW")
                nc.vector.scalar_tensor_tensor(
                    UW[:, 0:16], thw[:], float(-8.0 * K2), rowA2w,
                    ALU.mult, ALU.add)
                nc.vector.scalar_tensor_tensor(
                    UW[:, 16:32], thw[:], float(4096.0 / 255.0), rowB2w,
                    ALU.mult, ALU.add)
                WI = rw.tile([16, 32], I32, name="WI", tag="WI")
                nc.vector.tensor_copy(WI[:], UW[:])
                WC = rw.tile([16, 32], F32, name="WC", tag="WC")
                nc.vector.tensor_copy(WC[:], WI[:])
                WG = rw.tile([16, 32], F32, name="WG", tag="WG")
                nc.vector.tensor_tensor(WG[:], WC[:], UW[:], ALU.is_gt)
                WP = rw.tile([16, 32], F32, name="WP", tag="WP")
                nc.vector.tensor_tensor(WP[:], WC[:], WG[:], ALU.subtract)
                pidx = psA.tile([128, 32], F32, name="pidx", tag="pidx")
                nc.tensor.matmul(pidx[:], eye16[:], WP[:])
                nc.vector.tensor_copy(idx16[:], pidx[:])
                # keep the PE clock ramped through the gather window; the
                # idx16 (bitcast) rhs pins these AFTER the idx is ready so the
                # scheduler cannot hoist them out of the gather window
                for w in range(30):
                    nc.tensor.matmul(pconv[0:DPG, 0:32], eyef[0:16, 0:DPG],
                                     WC[:], skip_group_check=True)
                if DEBUG:
                    didx = wk.tile([16, 32], F32, name="didx", tag="didx")
                    nc.vector.tensor_copy(didx[:], idx16[0:16, :])
                    nc.sync.dma_start(dbg["dbg_idx"].ap(), didx[:])

                # ---------- gathers (SWDGE): kv first (unblocks k/v/psim) ----
                nc.gpsimd.dma_gather(
                    kvg[:].rearrange("p (b e) -> p b e", b=2),
                    din["xt2"].ap(), idx16[:, 16:32], NDS, NDS, 2 * DPG)
                nc.gpsimd.dma_gather(
                    cpbg[:].rearrange("p (b e) -> p b e", b=2),
                    din["cpb_tab"].ap(), idx16[:, 0:16], NDS, NDS, 2 * WIN)

            # psA (conv/proj/idx-phase PSUM) is closed here; kv needs its own
            with tc.tile_pool(name="psB", bufs=1, space="PSUM") as psB:
                # ---------- kv lerp + transpose + k/v, pipelined per H ----
                kvT = wk.tile([128, 128], F32, name="kvT", tag="kvT")
                for H in range(2):
                    b = 2 * DPG * H
                    nc.vector.tensor_tensor(
                        kvT[:, 64 * H:64 * H + 64],
                        kvg[:, b + DPG:b + 2 * DPG], kvg[:, b:b + DPG],
                        ALU.subtract)
                    nc.vector.scalar_tensor_tensor(
                        kvT[:, 64 * H:64 * H + 64],
                        kvT[:, 64 * H:64 * H + 64], fw[:, H:H + 1],
                        kvg[:, b:b + DPG], ALU.mult, ALU.add)
                pvTs = []
                for H in range(2):
                    cp = (nc.vector.tensor_copy if H == 0 else
                          (lambda o, i: nc.scalar.copy(o, i)))
                    pkv = psB.tile([DPG, 128], F32, name="pkv", tag=f"pkv{H}")
                    nc.tensor.transpose(pkv[:], kvT[:, 64 * H:64 * H + 64],
                                        eyef[:])
                    cp(kv_sb[:, 128 * H:128 * (H + 1)], pkv[:])
                    pkh = psB.tile([DPG, 128], F32, name="pk", tag=f"pk{H}")
                    nc.tensor.matmul(pkh[:], wkTs,
                                     kv_sb[:, 128 * H:128 * (H + 1)])
                    cp(k_sb[:, 128 * H:128 * (H + 1)], pkh[:])
                    pvT = psB.tile([128, DPG], F32, name="pvT", tag=f"pvT{H}")
                    nc.tensor.matmul(pvT[:], kv_sb[:, 128 * H:128 * (H + 1)], wvT)
                    pvTs.append(pvT)
                # vT copies on DVE (not needed until pav; keeps ACT free for exps)
                for H in range(2):
                    nc.vector.tensor_copy(vT[H][:], pvTs[H][:])
                if DEBUG:
                    nc.sync.dma_start(dbg["dbg_kv"].ap(), kv_sb[:].bitcast(F32))
                    nc.sync.dma_start(dbg["dbg_k"].ap(), k_sb[:].bitcast(F32))

            # ---------- attention ----------
            with (
                tc.tile_pool(name="epp", bufs=1) as epp,
                tc.tile_pool(name="psS", bufs=1, space="PSUM") as psS,
                tc.tile_pool(name="psY", bufs=1, space="PSUM") as psY,
                tc.tile_pool(name="psE", bufs=2, space="PSUM") as psE,
            ):
                psims = {}
                eps = {}
                for H in range(2):
                    for h in range(2):
                        ps = psS.tile([128, QS], F32, name="psim", tag=f"psim{h}")
                        nc.tensor.matmul(
                            ps[:], k_sb[32 * h:32 * (h + 1), 128 * H:128 * (H + 1)],
                            qs_sb[32 * h:32 * (h + 1), :])
                        psims[(h, H)] = ps
                        # exp(sim) on ACT right away (PSUM -> SBUF)
                        ep = epp.tile([128, QS], BF16, name=f"ep{h}{H}",
                                      tag=f"ep{h}{H}")
                        nc.scalar.activation(ep[:], ps[:], AF.Exp)
                        eps[(h, H)] = ep

                # table holds exp(G_o - C_o) on the delta/2 grid; nearest-
                # neighbor read (stride 2 along q): numer = exp(psim) * E.
                # Emission interleaves the reciprocals into the et chain so
                # DVE work overlaps PE sums and Pool broadcasts.
                ets, psums, pavs, rss, rsbs = {}, {}, {}, {}, {}

                def emit_et(h, H):
                    o = h
                    base = 2 * WIN * H + WIN * o
                    Rn = cpbg[:, base:base + QS]
                    et = wk.tile([128, QS], BF16, name=f"et{h}{H}",
                                 tag=f"et{h}{H}")
                    nc.vector.tensor_tensor(et[:], Rn, eps[(h, H)][:], ALU.mult)
                    ets[(h, H)] = et

                def emit_sums(h):
                    psum_s = psE.tile([1, QS], F32, name="psum_s", tag="psum_s")
                    for H in range(2):
                        nc.tensor.matmul(psum_s[:], ones_col[:], ets[(h, H)][:],
                                         start=(H == 0), stop=(H == 1))
                    psums[h] = psum_s
                    pav = psE.tile([32, QS], F32, name="pav", tag="pav")
                    for H in range(2):
                        nc.tensor.matmul(pav[:], vT[H][:, 32 * h:32 * (h + 1)],
                                         ets[(h, H)][:],
                                         start=(H == 0), stop=(H == 1))
                    pavs[h] = pav

                def emit_recip(h):
                    rs = rw.tile([1, QS], F32, name="rs", tag=f"rs{h}")
                    nc.vector.reciprocal(rs[:], psums[h][:])
                    rsb = wk.tile([32, QS], F32, name="rsb", tag=f"rsb{h}")
                    nc.gpsimd.partition_broadcast(rsb[:], rs[:])
                    rsbs[h] = rsb

                emit_et(0, 0)
                emit_et(1, 0)
                emit_et(0, 1)
                emit_sums(0)
                emit_recip(0)
                emit_et(1, 1)
                emit_sums(1)
                emit_recip(1)
                for h in range(2):
                    nc.vector.tensor_tensor(avn[32 * h:32 * (h + 1), :],
                                            pavs[h][:], rsbs[h][:], ALU.mult)
                if DEBUG:
                    nc.sync.dma_start(dbg["dbg_avn"].ap(), avn[:].bitcast(F32))

                # ---------- output projection (h-split accumulation so py
                # starts right after head 0's avn; fp16 output halves the DMA)
                pys = [psY.tile([128, QS], F32, name=f"py{m}", tag=f"py{m}")
                       for m in range(2)]
                for h in range(2):
                    for m in range(2):
                        nc.tensor.matmul(
                            pys[m][:],
                            woT[32 * h:32 * (h + 1), 128 * m:128 * (m + 1)],
                            avn[32 * h:32 * (h + 1), :],
                            start=(h == 0), stop=(h == 1))
                y_sb = wk.tile([128, 2 * QS], F16, name="y_sb", tag="y_sb")
                nc.scalar.copy(y_sb[:, 0:QS], pys[0][:])
                nc.vector.tensor_copy(y_sb[:, QS:2 * QS], pys[1][:])
                nc.sync.dma_start(y_out.ap(), y_sb[:])

    nc.compile()
    return nc


def _build_cpb_table(w1, b1, w2, b2, w3):
    """Windowed fp16 table of exp(G_o(pos) - C_o) on the delta/2 grid (the
    per-o shift C_o cancels in softmax; exp-space lets the bias apply as a
    multiply after exp(sim)). Returns [PMAX, 2*WIN] fp16."""
    m = np.arange(TLEN, dtype=np.float64)
    pos = POS0 + m * (1.0 / 1023.0)
    t = np.sign(pos) * np.log1p(np.abs(pos))
    H1 = np.maximum(t[:, None] * w1[None, :] + b1[None, :], 0.0)
    H2 = np.maximum(H1 @ w2.T + b2[None, :], 0.0)
    B = H2 @ w3.T                                        # [TLEN, 2] (b3 dropped)
    E = np.exp(B - B.max(axis=0, keepdims=True))
    E = np.maximum(E, 6.2e-5)   # keep fp16 normal; only where attn weight ~0
    # row p holds E[p], E[p+2], ..., E[p+2*(WIN-1)] per o (queries read
    # stride-2 on the delta/2 grid; pre-slicing makes device reads stride-1)
    sw = np.lib.stride_tricks.sliding_window_view(E, 2 * WIN, axis=0)
    sw = sw[:PMAX, :, ::2]                               # [PMAX, 2, WIN]
    return np.ascontiguousarray(
        sw.reshape(PMAX, 2 * WIN)).astype(np.float16)


def _shard_inputs(inputs):
    x = np.ascontiguousarray(inputs["x"][0]).astype(np.float32)   # [256, 1024]
    wq, wk, wv = inputs["wq"], inputs["wk"], inputs["wv"]
    wo = inputs["wo"]
    w_off_dw = inputs["w_off_dw"][:, 0, :]                 # [64, 6]
    b_off_dw = inputs["b_off_dw"]
    w_off_proj = inputs["w_off_proj"]
    w1 = np.asarray(inputs["cpb_w1"][:, 0], np.float64)
    b1 = np.asarray(inputs["cpb_b1"], np.float64)
    w2 = np.asarray(inputs["cpb_w2"], np.float64)
    b2 = np.asarray(inputs["cpb_b2"], np.float64)
    w3 = np.asarray(inputs["cpb_w3"], np.float64)

    f = np.float32
    cpb_tab = _build_cpb_table(w1, b1, w2, b2, w3)

    j = np.arange(NDS, dtype=np.float64)
    rowB2 = (1024.0 / 255.0) * j + 16.5

    in_maps = []
    for c in range(NCORES):
        g, qh = c // 2, c % 2
        xg = np.ascontiguousarray(x[64 * g:64 * (g + 1)], dtype=f)
        qbase = float(QS * qh)
        rowA2 = 2.0 * (qbase - POS0 * 511.5 - K2 * j) + 0.5
        rows2 = np.zeros((16, 288), np.float64)
        rows2[:, 0:16] = rowA2.reshape(16, 16).T
        rows2[:, 16:32] = rowB2.reshape(16, 16).T
        rows2[0, 32:288] = rowB2

        pka = np.zeros((DPG, PKA_C), f)
        for k in range(OFF_K):
            pka[:, 64 * k:64 * k + 64] = wq[g].T * w_off_dw[None, :, k]
        pka[:, 384:448] = wq[g].T
        pka[:, 448] = 0.5 * w_off_proj
        pka[0, 449:513] = b_off_dw
        pkb = np.zeros((DPG, PKB_C), f)
        pkb[:, 0:64] = wk[g].T * f(DH) ** f(-0.5)
        pkb[:, 64:128] = wv[g].T
        pkb[:, 128:384] = wo[:, 64 * g:64 * (g + 1)].T

        xpad = np.zeros((XROWS + 1, DPG), f)
        xpad[17:17 + N] = xg.T
        xt2 = np.concatenate([xpad[:-1], xpad[1:]], axis=1)  # [1059, 128]

        in_maps.append({
            "xg": xg,
            "rows2": rows2.astype(f),
            "xq": np.ascontiguousarray(xg[:, QS * qh:QS * (qh + 1)]),
            "packed_a": pka,
            "packed_b": pkb,
            "cpb_tab": cpb_tab,
            "xt2": np.ascontiguousarray(xt2),
        })
    return in_maps


def kernel(**inputs):
    if "nc" not in _CACHED:
        _CACHED["nc"] = build_nc()
    nc = _CACHED["nc"]
    in_maps = _shard_inputs(inputs)
    res = bass_utils.run_bass_kernel_spmd(nc, in_maps, core_ids=list(range(NCORES)))
    ys = [np.concatenate([res.results[c]["y"][:, 0:QS],
                          res.results[c]["y"][:, QS:2 * QS]], axis=0)
          for c in range(NCORES)]
    bo = inputs["bo"]
    out = np.zeros((1, DIM, N), np.float32)
    for qh in range(2):
        acc = np.zeros((DIM, QS), np.float64)
        for g in range(G):
            acc += ys[2 * g + qh]
        out[0, :, QS * qh:QS * (qh + 1)] = (
            acc + bo.astype(np.float64)[:, None]).astype(np.float32)
    return out


# revision 49
# speedup vs baseline: 1.0226x; 1.0226x over previous
"""DeformableAttention1D on 8 TRN2 NeuronCores via Bass/Tile.

Sharding: core c handles offset-group g=c//2 (64 of 256 channels, 2 of 8 heads)
and query-half qh=c%2 (512 of 1024 positions). Each core computes its group's
offsets/gather/CPB/attention independently; the final output projection is
computed as a partial (wo sliced by group) and summed on the host.

Key idea vs the one-hot/MLP baseline: both the grid_sample gather AND the CPB
relative-position-bias MLP are evaluated via SWDGE dma_gather from
host-precomputed DRAM tables.

  * kv gather: rows of x^T (zero-padded, pairs [x_i | x_{i+1}]) indexed by
    floor(pixel coord); bilinear lerp is 2 DVE ops with per-partition weights.
  * CPB bias: bias(q,j,o) = G_o(pos) with pos = grid_q[q] - vgs[j] and G_o a
    fixed scalar function of the CPB weights only. grid_q is a uniform grid
    with spacing delta = 2/1023, so for fixed j the 512 query positions read a
    CONTIGUOUS window of a delta-spaced table of G_o. One dma_gather of 256
    windowed rows (fp16) + a per-partition lerp replaces the whole MLP.
    (b3 is dropped: constant per (o,q) shift cancels in softmax.)

The ACT engine is restricted to ONE table set (exp_and_others: Exp, Tanh,
Square, Copy, Relu, ...); gelu uses the tanh approximation natively.
"""
import os
import sys

sys.path.insert(0, "/opt/trn_rl_repo")

DEBUG = bool(os.environ.get("DEFORM_DEBUG"))

import numpy as np

import concourse.bacc as bacc
import concourse.bass as bass
import concourse.mybir as mybir
import concourse.tile as tile
import concourse.bass_utils as bass_utils

F32 = mybir.dt.float32
F32R = mybir.dt.float32r
F16 = mybir.dt.float16
BF16 = mybir.dt.bfloat16
I32 = mybir.dt.int32
I16 = mybir.dt.int16
U32 = mybir.dt.uint32
AF = mybir.ActivationFunctionType
ALU = mybir.AluOpType

# model dims (hardcoded per problem spec)
DIM = 256
N = 1024
G = 4
HEADS = 8
DH = 32
NDS = 256          # downsampled kv positions
QS = 512           # queries per core
DPG = 64           # channels per group
OFF_K = 6
DS = 4             # downsample stride
OFF_SCALE = 4.0
NCORES = 8

DELTA = 2.0 / 1023.0
POS0 = -2.05
K2 = 1023.0 / 255.0
WIN = 512          # CPB table row length (stride-2 slice of delta/2 grid)
PMAX = 3200        # CPB windowed-table rows
TLEN = PMAX + 2 * WIN  # underlying table length (delta/2 spacing)
XROWS = 1059       # kv table rows (pairs), indexed by floor(ppix)+17

# A&S 7.1.26 erf coefficients (|err| <= 1.5e-7)
ERF_P = 0.3275911
ERF_A = [0.254829592, -0.284496736, 1.421413741, -1.453152027, 1.061405429]

# packed_a (f32r, [64, 516], conv-critical): wtaps 0:384, wqT 384:448,
#   wproj 448:449, bodw row (p0) 449:513.
# packed_b (f32r, [64, 384]): wkTs 0:64, wvT 64:128, woT 128:384.
# rowA2c/rowB2n (index affine rows) ship via the separate [1,512] "rows2"
# input (DVE TSP requires equal base partitions for its SB tensor inputs).
PKA_C = 516
PKB_C = 384

_CACHED = {}


def _patch_act_tables():
    """Restrict activation-table selection to the single set that covers all
    ACT functions used by this kernel, so exactly one table load is emitted."""
    import concourse.hw_specs as hw_specs

    if getattr(bacc, "_deform_act_patch", False):
        return
    orig = hw_specs.get_activation_tables

    keep = "exp_and_others"

    def patched(module_arch):
        tabs = orig(module_arch)
        keep_funcs = tabs[keep]
        out = {}
        for name, funcs in tabs.items():
            if name == keep:
                out[name] = funcs
            else:
                out[name] = funcs - keep_funcs
        return out

    bacc.get_activation_tables = patched
    bacc._deform_act_patch = True


def build_nc():
    _patch_act_tables()
    nc = bacc.Bacc("TRN2", target_bir_lowering=False, debug=False, num_devices=NCORES)

    din = {}

    def dt_in(name, shape, dtype=F32):
        din[name] = nc.dram_tensor(name, shape, dtype, kind="ExternalInput")
        return din[name]

    dt_in("xg", [DPG, N], F32R)
    dt_in("xq", [DPG, QS], F32R)
    dt_in("packed_a", [DPG, PKA_C], F32R)
    dt_in("packed_b", [DPG, PKB_C], F32R)
    dt_in("rows2", [16, 288], F32)
    dt_in("cpb_tab", [PMAX, 2 * WIN], F16)
    dt_in("xt2", [XROWS, 2 * DPG], F32)
    idx_scr = nc.dram_tensor("idx_scr", [1, NDS], F32, kind="Internal")
    y_out = nc.dram_tensor("y", [128, 2 * QS], F16, kind="ExternalOutput")
    dbg = {}
    if DEBUG:
        for nm, shp in [("dbg_conv", [DPG, NDS]), ("dbg_gl", [DPG, NDS]),
                        ("dbg_r", [1, NDS]), ("dbg_T2", [1, 2 * NDS]),
                        ("dbg_P2", [1, 2 * NDS]), ("dbg_idx", [16, 32]),
                        ("dbg_kv", [DPG, NDS]), ("dbg_k", [DPG, NDS]),
                        ("dbg_bias00", [128, QS]), ("dbg_logit00", [128, QS]),
                        ("dbg_avn", [DPG, QS])]:
            dbg[nm] = nc.dram_tensor(nm, shp, F32, kind="ExternalOutput")

    qh_off = 1  # xgp column offset of x (left zero pad)

    with tile.TileContext(nc) as tc:
        with (
            tc.tile_pool(name="const", bufs=1) as cst,
            tc.tile_pool(name="work", bufs=2) as wk,
            tc.tile_pool(name="rows", bufs=1) as rw,
            tc.tile_pool(name="pers", bufs=1) as pe_pool,
        ):
            # ---------- t=0: idle-engine prep ----------
            xgp = cst.tile([DPG, N + 4], F32R, name="xgp", tag="xgp")
            nc.gpsimd.memset(xgp[:, 0:1].bitcast(F32), 0.0)
            nc.gpsimd.memset(xgp[:, 1 + N:N + 4].bitcast(F32), 0.0)
            idx16 = cst.tile([128, 32], I16, name="idx16", tag="idx16")
            # tiled identity [16, 128]: eye16[c, j] = (j % 16 == c), for
            # replicating the idx block to all 8 Q7 16-partition groups
            eyeio16 = cst.tile([16, 128], I32, name="eyeio16", tag="eyeio16")
            nc.gpsimd.iota(eyeio16[:], pattern=[[0, 8], [1, 16]], base=0,
                           channel_multiplier=-1)
            eye16 = cst.tile([16, 128], F32, name="eye16", tag="eye16")
            nc.vector.tensor_scalar(eye16[:], eyeio16[:], 0, None, ALU.is_equal)
            ones_row = cst.tile([1, NDS], F32R, name="ones_row", tag="ones_row")
            nc.gpsimd.memset(ones_row[:].bitcast(F32), 1.0)
            ones_colf = cst.tile([128, 1], F32, name="ones_colf", tag="ones_colf")
            nc.gpsimd.memset(ones_colf[:], 1.0)
            ones_col = cst.tile([128, 1], BF16, name="ones_col", tag="ones_col")
            nc.vector.tensor_copy(ones_col[:], ones_colf[:])
            # identity for PE transposes (f32)
            eyeio = cst.tile([128, 128], I32, name="eyeio", tag="eyeio")
            nc.gpsimd.iota(eyeio[:], pattern=[[1, 128]], base=0, channel_multiplier=-1)
            eyef = cst.tile([128, 128], F32, name="eyef", tag="eyef")
            nc.vector.tensor_scalar(eyef[:], eyeio[:], 0, None, ALU.is_equal)
            # warm the single ACT table at t=0 (overlaps input DMAs)
            wsrc = cst.tile([128, 1], F32, name="wsrc", tag="wsrc")
            nc.gpsimd.memset(wsrc[:], 0.0)
            warm = cst.tile([128, 1], F32, name="warm", tag="warm")
            nc.scalar.activation(warm[:], wsrc[:], AF.Relu)
            # PE p-state warmup fodder
            wmm = cst.tile([128, 128], F32R, name="wmm", tag="wmm")
            nc.gpsimd.memset(wmm[:].bitcast(F32), 0.0)

            # ---------- input DMAs (packed on ACT queue; rest on SP) ----
            packed_a = cst.tile([DPG, PKA_C], F32R, name="packed_a", tag="packed_a")
            packed_b = cst.tile([DPG, PKB_C], F32R, name="packed_b", tag="packed_b")
            with tc.high_priority():
                nc.scalar.dma_start(packed_a[:], din["packed_a"].ap())
                nc.sync.dma_start(xgp[:, qh_off:qh_off + N], din["xg"].ap())
            nc.scalar.dma_start(packed_b[:], din["packed_b"].ap())
            xqt = cst.tile([DPG, QS], F32R, name="xqt", tag="xqt")
            nc.sync.dma_start(xqt[:], din["xq"].ap())
            rows2 = cst.tile([16, 288], F32, name="rows2", tag="rows2")
            nc.sync.dma_start(rows2[:], din["rows2"].ap())
            wtaps = packed_a[0:DPG, 0:384]
            wqT = packed_a[0:DPG, 384:448]
            wproj = packed_a[0:DPG, 448:449]
            bodw_row = packed_a[0:1, 449:513]
            wkTs = packed_b[0:DPG, 0:64]
            wvT = packed_b[0:DPG, 64:128]
            woT = packed_b[0:DPG, 128:384]
            rowA2w = rows2[0:16, 0:16]
            rowB2w = rows2[0:16, 16:32]
            rowB2r = rows2[0:1, 32:288]

            # persistent tiles crossing phases
            qs_sb = pe_pool.tile([DPG, QS], F32R, name="qs_sb", tag="qs_sb")
            k_sb = pe_pool.tile([DPG, NDS], F32R, name="k_sb", tag="k_sb")
            kv_sb = pe_pool.tile([DPG, NDS], F32R, name="kv_sb", tag="kv_sb")
            vT = [pe_pool.tile([128, DPG], BF16, name=f"vT{H}", tag=f"vT{H}")
                  for H in range(2)]
            fw = pe_pool.tile([128, 2], F32, name="fw", tag="fw")
            cpbg = pe_pool.tile([128, 2 * 2 * WIN], F16, name="cpbg", tag="cpbg")
            kvg = pe_pool.tile([128, 2 * 2 * DPG], F32, name="kvg", tag="kvg")
            avn = pe_pool.tile([DPG, QS], F32R, name="avn", tag="avn")

            with tc.tile_pool(name="psA", bufs=1, space="PSUM") as psA:
                # ---------- conv (strided depthwise fused with wq) ----------
                pconv = psA.tile([DPG, NDS], F32, name="pconv", tag="pconv")
                # PE clock warmup: dependency-free matmuls keep the ramp model
                # at full speed by the time real matmuls arrive
                for w in range(16):
                    nc.tensor.matmul(pconv[0:DPG, 0:64], wmm[:, 0:DPG],
                                     wmm[:, 0:64], skip_group_check=True)
                for k in range(OFF_K):
                    nc.tensor.matmul(
                        pconv[:], wtaps[:, 64 * k:64 * k + 64],
                        xgp[:, k:k + DS * (NDS - 1) + 1:DS],
                        start=(k == 0), stop=False)
                nc.tensor.matmul(pconv[:], bodw_row, ones_row[:],
                                 start=False, stop=True)
                if DEBUG:
                    dcv = wk.tile([DPG, NDS], F32, name="dcv", tag="dcv")
                    nc.vector.tensor_copy(dcv[:], pconv[:])
                    nc.sync.dma_start(dbg["dbg_conv"].ap(), dcv[:])

                # ---------- gelu (tanh approx, native ACT tanh) ----------
                # 2*gelu(x) = x * (1 + tanh(c1*(x + c2*x^3)))
                sq = wk.tile([DPG, NDS], F32, name="g_sq", tag="g_sq")
                nc.scalar.activation(sq[:], pconv[:], AF.Square)
                x3 = wk.tile([DPG, NDS], F32, name="g_x3", tag="g_x3")
                nc.vector.tensor_tensor(x3[:], sq[:], pconv[:], ALU.mult)
                arg = wk.tile([DPG, NDS], F32, name="g_arg", tag="g_arg")
                nc.vector.scalar_tensor_tensor(arg[:], x3[:], 0.044715, pconv[:],
                                               ALU.mult, ALU.add)
                tg = wk.tile([DPG, NDS], F32, name="g_tg", tag="g_tg")
                nc.scalar.activation(tg[:], arg[:], AF.Tanh,
                                     scale=0.7978845608028654)
                gl = wk.tile([DPG, NDS], F32R, name="g_gl", tag="g_gl")
                nc.vector.scalar_tensor_tensor(gl[:], tg[:], 1.0, pconv[:],
                                               ALU.add, ALU.mult)
                if DEBUG:
                    nc.sync.dma_start(dbg["dbg_gl"].ap(), gl[:].bitcast(F32))

                # ---------- proj + tanh (as r = 1/(e^{2p}+1)) ----------
                pproj = psA.tile([1, NDS], F32, name="pproj", tag="pproj")
                nc.tensor.matmul(pproj[:], wproj, gl[:])
                # qs early on PE (data ready; overlaps the row chain)
                pqs = psA.tile([DPG, QS], F32, name="pqs", tag="pqs")
                nc.tensor.matmul(pqs[:], wqT, xqt[:])
                nc.scalar.copy(qs_sb[:], pqs[:])

                th = rw.tile([1, NDS], F32, name="th", tag="th")
                nc.scalar.activation(th[:], pproj[:], AF.Tanh)
                # fire the tanh-row wrap round-trip immediately (SP queue)
                nc.sync.dma_start(idx_scr.ap(), th[:])
                thw = rw.tile([16, 16], F32, name="thw", tag="thw")
                nc.sync.dma_start(
                    thw[:], idx_scr.ap().rearrange("a (s p) -> (a p) s", p=16))
                if DEBUG:
                    nc.sync.dma_start(dbg["dbg_r"].ap(), th[:])

                # row side (overlapped): ppix row -> floor -> fraction -> fw
                PXr = rw.tile([1, NDS], F32, name="PXr", tag="PXr")
                nc.vector.scalar_tensor_tensor(
                    PXr[:], th[:], float(4096.0 / 255.0), rowB2r,
                    ALU.mult, ALU.add)
                XI = rw.tile([1, NDS], I32, name="XI", tag="XI")
                nc.vector.tensor_copy(XI[:], PXr[:])
                XC = rw.tile([1, NDS], F32, name="XC", tag="XC")
                nc.vector.tensor_copy(XC[:], XI[:])
                XG = rw.tile([1, NDS], F32, name="XG", tag="XG")
                nc.vector.tensor_tensor(XG[:], XC[:], PXr[:], ALU.is_gt)
                XP = rw.tile([1, NDS], F32, name="XP", tag="XP")
                nc.vector.tensor_tensor(XP[:], XC[:], XG[:], ALU.subtract)
                F2 = rw.tile([1, NDS], F32, name="F2", tag="F2")
                nc.vector.tensor_tensor(F2[:], PXr[:], XP[:], ALU.subtract)

                # kv lerp weights to per-partition columns: fw = [w1_H0, w1_H1]
                ptf = psA.tile([128, 2], F32, name="ptf", tag="ptf")
                for H in range(2):
                    nc.tensor.transpose(ptf[:, H:H + 1],
                                        F2[0:1, 128 * H:128 * (H + 1)],
                                        eyef[0:1, 0:1])
                nc.scalar.copy(fw[:], ptf[:])

                # wrapped side: u2/ppix affine on [16, 16], floors on [16, 32],
                # then replicate to all 8 Q7 core groups via PE matmul
                UW = rw.tile([16, 32], F32, name="UW", tag="UW")
                nc.vector.scalar_tensor_tensor(
                    UW[:, 0:16], thw[:], float(-8.0 * K2), rowA2w,
                    ALU.mult, ALU.add)
                nc.vector.scalar_tensor_tensor(
                    UW[:, 16:32], thw[:], float(4096.0 / 255.0), rowB2w,
                    ALU.mult, ALU.add)
                WI = rw.tile([16, 32], I32, name="WI", tag="WI")
                nc.vector.tensor_copy(WI[:], UW[:])
                WC = rw.tile([16, 32], F32, name="WC", tag="WC")
                nc.vector.tensor_copy(WC[:], WI[:])
                WG = rw.tile([16, 32], F32, name="WG", tag="WG")
                nc.vector.tensor_tensor(WG[:], WC[:], UW[:], ALU.is_gt)
                WP = rw.tile([16, 32], F32, name="WP", tag="WP")
                nc.vector.tensor_tensor(WP[:], WC[:], WG[:], ALU.subtract)
                pidx = psA.tile([128, 32], F32, name="pidx", tag="pidx")
                nc.tensor.matmul(pidx[:], eye16[:], WP[:])
                nc.vector.tensor_copy(idx16[:], pidx[:])
                # keep the PE clock ramped through the gather window; the
                # idx16 (bitcast) rhs pins these AFTER the idx is ready so the
                # scheduler cannot hoist them out of the gather window
                for w in range(30):
                    nc.tensor.matmul(pconv[0:DPG, 0:32], eyef[0:16, 0:DPG],
                                     WC[:], skip_group_check=True)
                if DEBUG:
                    didx = wk.tile([16, 32], F32, name="didx", tag="didx")
                    nc.vector.tensor_copy(didx[:], idx16[0:16, :])
                    nc.sync.dma_start(dbg["dbg_idx"].ap(), didx[:])

                # ---------- gathers (SWDGE): kv first (unblocks k/v/psim) ----
                nc.gpsimd.dma_gather(
                    kvg[:].rearrange("p (b e) -> p b e", b=2),
                    din["xt2"].ap(), idx16[:, 16:32], NDS, NDS, 2 * DPG)
                nc.gpsimd.dma_gather(
                    cpbg[:].rearrange("p (b e) -> p b e", b=2),
                    din["cpb_tab"].ap(), idx16[:, 0:16], NDS, NDS, 2 * WIN)

            # psA (conv/proj/idx-phase PSUM) is closed here; kv needs its own
            with tc.tile_pool(name="psB", bufs=1, space="PSUM") as psB:
                # ---------- kv lerp + transpose + k/v, pipelined per H ----
                kvT = wk.tile([128, 128], F32, name="kvT", tag="kvT")
                for H in range(2):
                    b = 2 * DPG * H
                    nc.vector.tensor_tensor(
                        kvT[:, 64 * H:64 * H + 64],
                        kvg[:, b + DPG:b + 2 * DPG], kvg[:, b:b + DPG],
                        ALU.subtract)
                    nc.vector.scalar_tensor_tensor(
                        kvT[:, 64 * H:64 * H + 64],
                        kvT[:, 64 * H:64 * H + 64], fw[:, H:H + 1],
                        kvg[:, b:b + DPG], ALU.mult, ALU.add)
                pvTs = []
                for H in range(2):
                    cp = (nc.vector.tensor_copy if H == 0 else
                          (lambda o, i: nc.scalar.copy(o, i)))
                    pkv = psB.tile([DPG, 128], F32, name="pkv", tag=f"pkv{H}")
                    nc.tensor.transpose(pkv[:], kvT[:, 64 * H:64 * H + 64],
                                        eyef[:])
                    cp(kv_sb[:, 128 * H:128 * (H + 1)], pkv[:])
                    pkh = psB.tile([DPG, 128], F32, name="pk", tag=f"pk{H}")
                    nc.tensor.matmul(pkh[:], wkTs,
                                     kv_sb[:, 128 * H:128 * (H + 1)])
                    cp(k_sb[:, 128 * H:128 * (H + 1)], pkh[:])
                    pvT = psB.tile([128, DPG], F32, name="pvT", tag=f"pvT{H}")
                    nc.tensor.matmul(pvT[:], kv_sb[:, 128 * H:128 * (H + 1)], wvT)
                    pvTs.append(pvT)
                # vT copies on DVE (not needed until pav; keeps ACT free for exps)
                for H in range(2):
                    nc.vector.tensor_copy(vT[H][:], pvTs[H][:])
                if DEBUG:
                    nc.sync.dma_start(dbg["dbg_kv"].ap(), kv_sb[:].bitcast(F32))
                    nc.sync.dma_start(dbg["dbg_k"].ap(), k_sb[:].bitcast(F32))

            # ---------- attention ----------
            with (
                tc.tile_pool(name="epp", bufs=1) as epp,
                tc.tile_pool(name="psS", bufs=1, space="PSUM") as psS,
                tc.tile_pool(name="psY", bufs=1, space="PSUM") as psY,
                tc.tile_pool(name="psE", bufs=2, space="PSUM") as psE,
            ):
                psims = {}
                eps = {}
                for H in range(2):
                    for h in range(2):
                        ps = psS.tile([128, QS], F32, name="psim", tag=f"psim{h}")
                        nc.tensor.matmul(
                            ps[:], k_sb[32 * h:32 * (h + 1), 128 * H:128 * (H + 1)],
                            qs_sb[32 * h:32 * (h + 1), :])
                        psims[(h, H)] = ps
                        # exp(sim) on ACT right away (PSUM -> SBUF)
                        ep = epp.tile([128, QS], BF16, name=f"ep{h}{H}",
                                      tag=f"ep{h}{H}")
                        nc.scalar.activation(ep[:], ps[:], AF.Exp)
                        eps[(h, H)] = ep

                # table holds exp(G_o - C_o) on the delta/2 grid; nearest-
                # neighbor read (stride 2 along q): numer = exp(psim) * E.
                # Emission interleaves the reciprocals into the et chain so
                # DVE work overlaps PE sums and Pool broadcasts.
                ets, psums, pavs, rss, rsbs = {}, {}, {}, {}, {}

                def emit_et(h, H):
                    o = h
                    base = 2 * WIN * H + WIN * o
                    Rn = cpbg[:, base:base + QS]
                    et = wk.tile([128, QS], BF16, name=f"et{h}{H}",
                                 tag=f"et{h}{H}")
                    nc.vector.tensor_tensor(et[:], Rn, eps[(h, H)][:], ALU.mult)
                    ets[(h, H)] = et

                def emit_sums(h):
                    psum_s = psE.tile([1, QS], F32, name="psum_s", tag="psum_s")
                    for H in range(2):
                        nc.tensor.matmul(psum_s[:], ones_col[:], ets[(h, H)][:],
                                         start=(H == 0), stop=(H == 1))
                    psums[h] = psum_s
                    pav = psE.tile([32, QS], F32, name="pav", tag="pav")
                    for H in range(2):
                        nc.tensor.matmul(pav[:], vT[H][:, 32 * h:32 * (h + 1)],
                                         ets[(h, H)][:],
                                         start=(H == 0), stop=(H == 1))
                    pavs[h] = pav

                def emit_recip(h):
                    rs = rw.tile([1, QS], F32, name="rs", tag=f"rs{h}")
                    nc.vector.reciprocal(rs[:], psums[h][:])
                    rsb = wk.tile([32, QS], F32, name="rsb", tag=f"rsb{h}")
                    nc.gpsimd.partition_broadcast(rsb[:], rs[:])
                    rsbs[h] = rsb

                emit_et(0, 0)
                emit_et(1, 0)
                emit_et(0, 1)
                emit_sums(0)
                emit_recip(0)
                emit_et(1, 1)
                emit_sums(1)
                emit_recip(1)
                for h in range(2):
                    nc.vector.tensor_tensor(avn[32 * h:32 * (h + 1), :],
                                            pavs[h][:], rsbs[h][:], ALU.mult)
                if DEBUG:
                    nc.sync.dma_start(dbg["dbg_avn"].ap(), avn[:].bitcast(F32))

                # ---------- output projection (h-split accumulation so py
                # starts right after head 0's avn; fp16 output halves the DMA)
                pys = [psY.tile([128, QS], F32, name=f"py{m}", tag=f"py{m}")
                       for m in range(2)]
                for h in range(2):
                    for m in range(2):
                        nc.tensor.matmul(
                            pys[m][:],
                            woT[32 * h:32 * (h + 1), 128 * m:128 * (m + 1)],
                            avn[32 * h:32 * (h + 1), :],
                            start=(h == 0), stop=(h == 1))
                y_sb = wk.tile([128, 2 * QS], F16, name="y_sb", tag="y_sb")
                nc.scalar.copy(y_sb[:, 0:QS], pys[0][:])
                nc.vector.tensor_copy(y_sb[:, QS:2 * QS], pys[1][:])
                nc.sync.dma_start(y_out.ap(), y_sb[:])

    nc.compile()
    return nc


def _build_cpb_table(w1, b1, w2, b2, w3):
    """Windowed fp16 table of exp(G_o(pos) - C_o) on the delta/2 grid (the
    per-o shift C_o cancels in softmax; exp-space lets the bias apply as a
    multiply after exp(sim)). Returns [PMAX, 2*WIN] fp16."""
    m = np.arange(TLEN, dtype=np.float64)
    pos = POS0 + m * (1.0 / 1023.0)
    t = np.sign(pos) * np.log1p(np.abs(pos))
    H1 = np.maximum(t[:, None] * w1[None, :] + b1[None, :], 0.0)
    H2 = np.maximum(H1 @ w2.T + b2[None, :], 0.0)
    B = H2 @ w3.T                                        # [TLEN, 2] (b3 dropped)
    E = np.exp(B - B.max(axis=0, keepdims=True))
    E = np.maximum(E, 6.2e-5)   # keep fp16 normal; only where attn weight ~0
    # row p holds E[p], E[p+2], ..., E[p+2*(WIN-1)] per o (queries read
    # stride-2 on the delta/2 grid; pre-slicing makes device reads stride-1)
    sw = np.lib.stride_tricks.sliding_window_view(E, 2 * WIN, axis=0)
    sw = sw[:PMAX, :, ::2]                               # [PMAX, 2, WIN]
    return np.ascontiguousarray(
        sw.reshape(PMAX, 2 * WIN)).astype(np.float16)


def _shard_inputs(inputs):
    x = np.ascontiguousarray(inputs["x"][0]).astype(np.float32)   # [256, 1024]
    wq, wk, wv = inputs["wq"], inputs["wk"], inputs["wv"]
    wo = inputs["wo"]
    w_off_dw = inputs["w_off_dw"][:, 0, :]                 # [64, 6]
    b_off_dw = inputs["b_off_dw"]
    w_off_proj = inputs["w_off_proj"]
    w1 = np.asarray(inputs["cpb_w1"][:, 0], np.float64)
    b1 = np.asarray(inputs["cpb_b1"], np.float64)
    w2 = np.asarray(inputs["cpb_w2"], np.float64)
    b2 = np.asarray(inputs["cpb_b2"], np.float64)
    w3 = np.asarray(inputs["cpb_w3"], np.float64)

    f = np.float32
    cpb_tab = _build_cpb_table(w1, b1, w2, b2, w3)

    j = np.arange(NDS, dtype=np.float64)
    rowB2 = (1024.0 / 255.0) * j + 16.5

    in_maps = []
    for c in range(NCORES):
        g, qh = c // 2, c % 2
        xg = np.ascontiguousarray(x[64 * g:64 * (g + 1)], dtype=f)
        qbase = float(QS * qh)
        rowA2 = 2.0 * (qbase - POS0 * 511.5 - K2 * j) + 0.5
        rows2 = np.zeros((16, 288), np.float64)
        rows2[:, 0:16] = rowA2.reshape(16, 16).T
        rows2[:, 16:32] = rowB2.reshape(16, 16).T
        rows2[0, 32:288] = rowB2

        pka = np.zeros((DPG, PKA_C), f)
        for k in range(OFF_K):
            pka[:, 64 * k:64 * k + 64] = wq[g].T * w_off_dw[None, :, k]
        pka[:, 384:448] = wq[g].T
        pka[:, 448] = 0.5 * w_off_proj
        pka[0, 449:513] = b_off_dw
        pkb = np.zeros((DPG, PKB_C), f)
        pkb[:, 0:64] = wk[g].T * f(DH) ** f(-0.5)
        pkb[:, 64:128] = wv[g].T
        pkb[:, 128:384] = wo[:, 64 * g:64 * (g + 1)].T

        xpad = np.zeros((XROWS + 1, DPG), f)
        xpad[17:17 + N] = xg.T
        xt2 = np.concatenate([xpad[:-1], xpad[1:]], axis=1)  # [1059, 128]

        in_maps.append({
            "xg": xg,
            "rows2": rows2.astype(f),
            "xq": np.ascontiguousarray(xg[:, QS * qh:QS * (qh + 1)]),
            "packed_a": pka,
            "packed_b": pkb,
            "cpb_tab": cpb_tab,
            "xt2": np.ascontiguousarray(xt2),
        })
    return in_maps


def kernel(**inputs):
    if "nc" not in _CACHED:
        _CACHED["nc"] = build_nc()
    nc = _CACHED["nc"]
    in_maps = _shard_inputs(inputs)
    res = bass_utils.run_bass_kernel_spmd(nc, in_maps, core_ids=list(range(NCORES)))
    ys = [np.concatenate([res.results[c]["y"][:, 0:QS],
                          res.results[c]["y"][:, QS:2 * QS]], axis=0)
          for c in range(NCORES)]
    bo = inputs["bo"]
    out = np.zeros((1, DIM, N), np.float32)
    for qh in range(2):
        acc = np.zeros((DIM, QS), np.float64)
        for g in range(G):
            acc += ys[2 * g + qh]
        out[0, :, QS * qh:QS * (qh + 1)] = (
            acc + bo.astype(np.float64)[:, None]).astype(np.float32)
    return out
